# revision 1
# baseline (speedup 1.0000x reference)
"""BiLSTM tagger + biaffine scorer on 8 Trainium2 NeuronCores (Bass/Tile).

Strategy
--------
- The 100k x 300 word-embedding table is sharded row-wise across the 8 cores
  (model-parallel gather, per the sharding hint). Each core gathers its owned
  rows via a transposed dma_gather (producing the feature-major x^T layout the
  matmuls want) and an AllReduce combines the partial gathers.
- The 2-layer BiLSTM is inherently sequential (1024 dependent cell steps);
  batch=1 gives nothing to shard, so every core runs it redundantly (fwd+bwd
  chains interleaved on each core so ACT/DVE latency hides under the PE weight
  stream). Recurrent matvecs keep gates partition-major: stationary = Whh
  tiles (bf16, 128-col tiles so the fast-weight-load path engages), moving =
  h (one column). Input projections for a 128-step window are batched matmuls
  accumulated directly into the gate PSUM banks (even/odd step parity in
  separate banks so TensorE writes never collide with ScalarE reads).
- Head/dep MLPs + biaffine score run feature-major end to end (zero
  transposes anywhere in the kernel: the gather, LSTM state writes, MLPs and
  the final score matmul all produce/consume [feature, token] layouts).
- Token columns are stored parity-blocked ([evens | odds], so window
  projections hit contiguous columns); the host un-permutes the output.
"""

import os
import sys

sys.path.insert(0, "/opt/trn_rl_repo")

import numpy as np
import ml_dtypes

import concourse.bass as bass
import concourse.tile as tile
from concourse import bacc, mybir
from concourse.bass_utils import run_bass_kernel_spmd

BF16 = ml_dtypes.bfloat16

N_CORES = 8
SEQ = int(os.environ.get("NN_SEQ", "512"))
HALF = SEQ // 2
NW = max(1, SEQ // 128)          # LSTM step windows of 128
WIN = SEQ // NW                  # steps per window (128 normally)
H = 200                          # hidden per direction
GS = 1024                        # padded gate slots (4 gates x 256)
V = 100000
VSH = V // N_CORES               # word rows per core
WCOLS = 384                      # padded word emb row (bf16)
PCOLS = 128                      # padded pos emb row (bf16)
NT = SEQ // 128                  # token tiles

F32 = mybir.dt.float32
BF = mybir.dt.bfloat16
I16 = mybir.dt.int16

AF = mybir.ActivationFunctionType

# gate block order in the padded layout: i, f, o, g  (sigmoid gates first)
_GATE_SRC = [0, 1, 3, 2]  # torch order is i, f, g, o


# ----------------------------------------------------------------------------
# host-side weight/index preparation (pure numpy layout transforms)
# ----------------------------------------------------------------------------

def _gate_pad(W):
    """[800, ...] torch-gate-ordered -> [1024, ...] (i,f,o,g) each padded to 256."""
    out = np.zeros((GS,) + W.shape[1:], np.float32)
    for b, s in enumerate(_GATE_SRC):
        out[b * 256 : b * 256 + H] = W[s * H : (s + 1) * H]
    return out


def _prep_wih1(Wih):
    """layer-1 input proj [800, 400] -> lhsT [512 in-slots, 1024]."""
    Wr = _gate_pad(Wih)                      # [1024, 400]
    p = np.zeros((512, GS), np.float32)
    p[0:300] = Wr[:, 0:300].T                # word feats -> slots 0..299
    p[384:484] = Wr[:, 300:400].T            # pos feats  -> slots 384..483
    return p.astype(BF16)


def _prep_wih2(Wih):
    """layer-2 input proj [800, 400] -> lhsT [512 in-slots, 1024]."""
    Wr = _gate_pad(Wih)
    p = np.zeros((512, GS), np.float32)
    p[0:200] = Wr[:, 0:200].T                # fwd feats -> slots 0..199
    p[256:456] = Wr[:, 200:400].T            # bwd feats -> slots 256..455
    return p.astype(BF16)


def _prep_whh(Whh):
    """[800, 200] -> lhsT [256 h-slots, 1024]."""
    Wr = _gate_pad(Whh)                      # [1024, 200]
    p = np.zeros((256, GS), np.float32)
    p[0:200] = Wr.T
    return p.astype(BF16)


def _prep_mlp_in_x2(W):
    """MLP weight [400 out, 400 in-of-x2] -> lhsT [512 x2-slots, 512 out-slots]."""
    p = np.zeros((512, 512), np.float32)
    p[0:200, 0:400] = W[:, 0:200].T
    p[256:456, 0:400] = W[:, 200:400].T
    return p


def _prep_mlp_in_h(W):
    """MLP weight [400 out, 400 in-of-h1] -> lhsT [512, 512]."""
    p = np.zeros((512, 512), np.float32)
    p[0:400, 0:400] = W.T
    return p


def _prep_wbi(W_bi):
    p = np.zeros((512, 512), np.float32)
    p[0:400, 0:400] = W_bi
    return p


def _wrap_idx(idx):
    """[SEQ] int -> [128, SEQ//16] int16 in the dma_gather wrapped layout."""
    n = idx.shape[0]
    a = np.zeros((16, n // 16), np.int16)
    for i, v in enumerate(idx):
        a[i % 16, i // 16] = v
    return np.tile(a, (8, 1))


def _pos_order():
    """parity-blocked token order: evens then odds; order[p] = token at col p."""
    return np.concatenate([np.arange(0, SEQ, 2), np.arange(1, SEQ, 2)])


def _pos_of(t):
    return (t % 2) * HALF + t // 2


# ----------------------------------------------------------------------------
# device program
# ----------------------------------------------------------------------------

def _build(b_bi_val, sim=False):
    nc = bacc.Bacc("TRN2", target_bir_lowering=False, debug=False,
                   num_devices=1 if sim else N_CORES)
    dt = mybir.dt

    def din(name, shape, d):
        return nc.dram_tensor(name, shape, d, kind="ExternalInput").ap()

    wtab = din("wtab", [VSH + 1, WCOLS], BF)
    ptab = din("ptab", [50, PCOLS], BF)
    widx = din("widx", [128, SEQ // 16], I16)
    pidx = din("pidx", [128, SEQ // 16], I16)
    wih = {(0, "f"): din("wih1f", [512, GS], BF),
           (0, "b"): din("wih1b", [512, GS], BF),
           (1, "f"): din("wih2f", [512, GS], BF),
           (1, "b"): din("wih2b", [512, GS], BF)}
    whh = {(0, "f"): din("whh1f", [256, GS], BF),
           (0, "b"): din("whh1b", [256, GS], BF),
           (1, "f"): din("whh2f", [256, GS], BF),
           (1, "b"): din("whh2b", [256, GS], BF)}
    gb = din("gb", [1, 4, GS], BF)
    R32d = mybir.dt.float32r
    wh1 = din("wh1", [512, 512], R32d)
    wh2 = din("wh2", [512, 512], R32d)
    wd1 = din("wd1", [512, 512], R32d)
    wd2 = din("wd2", [512, 512], R32d)
    wbi = din("wbi", [512, 512], R32d)
    mb = din("mb", [1, 4, 512], R32d)
    out = nc.dram_tensor("out", [SEQ, SEQ], F32, kind="ExternalOutput").ap()

    arw_in = nc.dram_tensor("arw_in", [128, 3 * SEQ], BF).ap()
    arw_out = nc.dram_tensor("arw_out", [128, 3 * SEQ], BF,
                             addr_space="Local" if sim else "Shared").ap()

    from contextlib import ExitStack

    with tile.TileContext(nc) as tc, ExitStack() as ctx:
        wp = ctx.enter_context(tc.tile_pool(name="w", bufs=1))
        sp = ctx.enter_context(tc.tile_pool(name="s", bufs=6))

        # ---- persistent SBUF tensors -------------------------------------
        def wtile(tag, shape, d):
            return wp.tile(shape, d, tag=tag, name=tag)

        xin = wtile("xin", [128, 4, SEQ], BF)        # layer-1 input x^T
        X1 = {"f": wtile("X1f", [128, 2, SEQ], BF),
              "b": wtile("X1b", [128, 2, SEQ], BF)}
        X2 = {"f": wtile("X2f", [128, 2, SEQ], BF),
              "b": wtile("X2b", [128, 2, SEQ], BF)}
        wih_sb = {k: wtile(f"wih{k}", [128, 4, GS], BF) for k in wih}
        whh_sb = {k: wtile(f"whh{k}", [128, 2, GS], BF) for k in whh}
        gb_sb = wtile("gb", [1, 4, GS], BF)
        R32 = mybir.dt.float32r
        mlp_sb = {n: wtile(n, [128, 4, 512], R32)
                  for n in ("wh1", "wh2", "wd1", "wd2", "wbi")}
        mb_sb = wtile("mb", [1, 4, 512], R32)
        ones = wtile("ones", [1, SEQ], BF)
        ones_f = wtile("ones_f", [1, SEQ], R32)
        zh = wtile("zh", [128, 1], BF)
        cst = {"f": wtile("cf", [128, 2], F32), "b": wtile("cb", [128, 2], F32)}
        X2F = {"f": wtile("X2Ff", [128, 2, SEQ], R32),
               "b": wtile("X2Fb", [128, 2, SEQ], R32)}
        h1T = wtile("h1T", [128, 4, SEQ], R32)
        headT = wtile("headT", [128, 4, SEQ], R32)
        depT = wtile("depT", [128, 4, SEQ], R32)
        AT = wtile("AT", [128, 4, SEQ], R32)
        S_sb = wtile("S", [128, NT, SEQ], F32)
        widx_sb = wtile("widx", [128, SEQ // 16], I16)
        pidx_sb = wtile("pidx", [128, SEQ // 16], I16)

        # ---- load weights ------------------------------------------------
        for k in wih:
            nc.sync.dma_start(out=wih_sb[k][:],
                              in_=wih[k].rearrange("(k p) c -> p k c", p=128))
        for k in whh:
            nc.sync.dma_start(out=whh_sb[k][:],
                              in_=whh[k].rearrange("(k p) c -> p k c", p=128))
        for n in ("wh1", "wh2", "wd1", "wd2", "wbi"):
            src = {"wh1": wh1, "wh2": wh2, "wd1": wd1, "wd2": wd2, "wbi": wbi}[n]
            nc.sync.dma_start(out=mlp_sb[n][:],
                              in_=src.rearrange("(k p) c -> p k c", p=128))
        nc.sync.dma_start(out=gb_sb[:], in_=gb[:])
        nc.sync.dma_start(out=mb_sb[:], in_=mb[:])
        nc.sync.dma_start(out=widx_sb[:], in_=widx[:])
        nc.sync.dma_start(out=pidx_sb[:], in_=pidx[:])
        nc.vector.memset(ones[:], 1.0)
        nc.vector.tensor_copy(ones_f[:], ones[:])
        nc.vector.memset(zh[:], 0.0)

        # ---- embedding gather (sharded word table + AllReduce) -----------
        nc.gpsimd.dma_gather(out_ap=xin[:, 0:3, :], in_ap=wtab[:],
                             idxs_ap=widx_sb[:], num_idxs=SEQ,
                             num_idxs_reg=SEQ, elem_size=WCOLS, transpose=True)
        nc.gpsimd.dma_gather(out_ap=xin[:, 3:4, :], in_ap=ptab[:],
                             idxs_ap=pidx_sb[:], num_idxs=SEQ,
                             num_idxs_reg=SEQ, elem_size=PCOLS, transpose=True)
        nc.sync.dma_start(out=arw_in[:], in_=xin[:, 0:3, :])
        if sim:
            nc.sync.dma_start(out=arw_out[:], in_=arw_in[:])
        else:
            nc.gpsimd.collective_compute(
                "AllReduce", mybir.AluOpType.add,
                replica_groups=[list(range(N_CORES))],
                ins=[arw_in[:]], outs=[arw_out[:]])
        nc.sync.dma_start(out=xin[:, 0:3, :], in_=arw_out[:])

        # ---- LSTM --------------------------------------------------------
        def in_chunks(l):
            if l == 0:
                return [xin[:, c, :] for c in range(4)]
            return [X1["f"][:, 0, :], X1["f"][:, 1, :],
                    X1["b"][:, 0, :], X1["b"][:, 1, :]]

        def proj_rhs_start(c, w, p):
            # column range start (64 cols) feeding (chain c, window w, parity p)
            if c == "f":
                return p * HALF + w * 64
            return (1 - p) * HALF + (HALF - 1 - w * 64) - 63

        def col_in_bank(c, i):
            w = i // WIN
            if c == "f":
                return i // 2 - w * 64
            jp = (i - w * WIN - (i % 2)) // 2
            return 63 - jp

        lsmctx = ExitStack()
        psum = lsmctx.enter_context(tc.tile_pool(name="psum", bufs=2, space="PSUM"))
        for l in (0, 1):
            chunks = in_chunks(l)
            Xout = X1 if l == 0 else X2
            for c in ("f", "b"):
                nc.vector.memset(cst[c][:], 0.0)
            banks = {}
            for w in range(NW):
                # window projections (both chains, both parities)
                for c in ("f", "b"):
                    gbrow = l * 2 + (0 if c == "f" else 1)
                    for p in (0, 1):
                        bank = psum.tile([128, 8, 64], F32, tag=f"g{c}{p}", name=f"g{c}{p}")
                        banks[(c, p)] = bank
                        rs = proj_rhs_start(c, w, p)
                        for m in range(8):
                            ms = slice(m * 128, (m + 1) * 128)
                            for k in range(4):
                                nc.tensor.matmul(
                                    out=bank[:, m, :],
                                    lhsT=wih_sb[(l, c)][:, k, ms],
                                    rhs=chunks[k][:, rs:rs + 64],
                                    start=(m == 0 and k == 0), stop=False,
                                    skip_group_check=True)
                            nc.tensor.matmul(
                                out=bank[:, m, :],
                                lhsT=gb_sb[0:1, gbrow, ms],
                                rhs=ones[:, 0:64],
                                start=False, stop=False, skip_group_check=True)
                # recurrence steps, chains interleaved
                for s in range(WIN):
                    i = w * WIN + s
                    for c in ("f", "b"):
                        t = i if c == "f" else SEQ - 1 - i
                        p = i % 2
                        bank = banks[(c, p)]
                        tl = col_in_bank(c, i)
                        last = (s >= WIN - 2)
                        if i == 0:
                            hs = [zh[:, 0:1], zh[:, 0:1]]
                        else:
                            tp = (i - 1) if c == "f" else (SEQ - i)
                            pc = _pos_of(tp)
                            hs = [Xout[c][:, k, pc:pc + 1] for k in (0, 1)]
                        for m in range(8):
                            ms = slice(m * 128, (m + 1) * 128)
                            for k in (0, 1):
                                nc.tensor.matmul(
                                    out=bank[:, m, tl:tl + 1],
                                    lhsT=whh_sb[(l, c)][:, k, ms],
                                    rhs=hs[k],
                                    start=False,
                                    stop=(last and m == 7 and k == 1),
                                    skip_group_check=True)
                        sg = sp.tile([128, 6], F32, tag=f"sg{c}", name=f"sg{c}")
                        nc.scalar.activation(sg[:], bank[:, 0:6, tl], AF.Sigmoid)
                        tg = sp.tile([128, 2], F32, tag=f"tg{c}", name=f"tg{c}")
                        nc.scalar.activation(tg[:], bank[:, 6:8, tl], AF.Tanh)
                        t1 = sp.tile([128, 2], F32, tag=f"t1{c}", name=f"t1{c}")
                        nc.vector.tensor_mul(t1[:], sg[:, 0:2], tg[:])
                        t2 = sp.tile([128, 2], F32, tag=f"t2{c}", name=f"t2{c}")
                        nc.vector.tensor_mul(t2[:], sg[:, 2:4], cst[c][:])
                        nc.vector.tensor_add(cst[c][:], t1[:], t2[:])
                        tcl = sp.tile([128, 2], F32, tag=f"tc{c}", name=f"tc{c}")
                        nc.scalar.activation(tcl[:], cst[c][:], AF.Tanh)
                        pw = _pos_of(t)
                        nc.vector.tensor_mul(Xout[c][:, :, pw],
                                             sg[:, 4:6], tcl[:])
        lsmctx.close()

        # ---- head/dep MLPs + biaffine ------------------------------------
        psum2 = ctx.enter_context(tc.tile_pool(name="psum2", bufs=4, space="PSUM"))
        for c in ("f", "b"):
            nc.vector.tensor_copy(X2F[c][:], X2[c][:])
        x2c = [X2F["f"][:, 0, :], X2F["f"][:, 1, :],
               X2F["b"][:, 0, :], X2F["b"][:, 1, :]]

        def mlp(dst, wname, brow, chunks):
            for mt in range(4):
                ms = slice(mt * 128, (mt + 1) * 128)
                ps = psum2.tile([128, SEQ], F32, tag="mlp", name="mlp")
                for k in range(4):
                    nc.tensor.matmul(out=ps[:],
                                     lhsT=mlp_sb[wname][:, k, ms],
                                     rhs=chunks[k],
                                     start=(k == 0), stop=False,
                                     skip_group_check=True)
                nc.tensor.matmul(out=ps[:], lhsT=mb_sb[0:1, brow, ms],
                                 rhs=ones_f[:], start=False, stop=True,
                                 skip_group_check=True)
                nc.scalar.activation(dst[:, mt, :], ps[:], AF.Relu)

        def tchunks(t):
            return [t[:, k, :] for k in range(4)]

        mlp(h1T, "wh1", 0, x2c)
        mlp(headT, "wh2", 1, tchunks(h1T))
        mlp(h1T, "wd1", 2, x2c)
        mlp(depT, "wd2", 3, tchunks(h1T))

        for mt in range(4):
            ms = slice(mt * 128, (mt + 1) * 128)
            ps = psum2.tile([128, SEQ], F32, tag="mlp", name="mlp")
            for k in range(4):
                nc.tensor.matmul(out=ps[:],
                                 lhsT=mlp_sb["wbi"][:, k, ms],
                                 rhs=headT[:, k, :], start=(k == 0),
                                 stop=(k == 3), skip_group_check=True)
            nc.vector.tensor_copy(AT[:, mt, :], ps[:])

        for mt in range(NT):
            ms = slice(mt * 128, (mt + 1) * 128)
            ps = psum2.tile([128, SEQ], F32, tag="mlp", name="mlp")
            for k in range(4):
                nc.tensor.matmul(out=ps[:], lhsT=AT[:, k, ms],
                                 rhs=depT[:, k, :], start=(k == 0),
                                 stop=(k == 3), skip_group_check=True)
            nc.vector.tensor_scalar_add(S_sb[:, mt, :], ps[:], b_bi_val)
            nc.sync.dma_start(out=out[mt * 128:(mt + 1) * 128, :],
                              in_=S_sb[:, mt, :])

    nc.compile()
    return nc


_NC_CACHE = {}


def _get_nc(b_bi_val):
    if b_bi_val not in _NC_CACHE:
        _NC_CACHE[b_bi_val] = _build(b_bi_val)
    return _NC_CACHE[b_bi_val]


# ----------------------------------------------------------------------------
# entry point
# ----------------------------------------------------------------------------

def _prep_in_maps(inputs):
    return _prep(**inputs)


def _prep(word_emb, pos_emb, Wih, Whh, bih, bhh,
          W_h1, b_h1, W_h2, b_h2, W_d1, b_d1, W_d2, b_d2,
          W_bi, b_bi, sentence_word_indices, sentence_pos_indices):
    order = _pos_order()
    widx_g = np.asarray(sentence_word_indices)[order].astype(np.int64)
    pidx_g = np.asarray(sentence_pos_indices)[order].astype(np.int64)

    wtab_full = np.zeros((V, WCOLS), np.float32)
    wtab_full[:, :300] = np.asarray(word_emb, np.float32)
    ptab = np.zeros((50, PCOLS), np.float32)
    ptab[:, :100] = np.asarray(pos_emb, np.float32)
    ptab = ptab.astype(BF16)

    if np.asarray(W_bi).ndim == 3:
        W_bi = np.asarray(W_bi)[0]

    base = {
        "ptab": ptab,
        "pidx": _wrap_idx(pidx_g),
        "wih1f": _prep_wih1(Wih[0, 0]), "wih1b": _prep_wih1(Wih[0, 1]),
        "wih2f": _prep_wih2(Wih[1, 0]), "wih2b": _prep_wih2(Wih[1, 1]),
        "whh1f": _prep_whh(Whh[0, 0]), "whh1b": _prep_whh(Whh[0, 1]),
        "whh2f": _prep_whh(Whh[1, 0]), "whh2b": _prep_whh(Whh[1, 1]),
        "gb": np.stack([_gate_pad((np.asarray(bih[l, d]) + np.asarray(bhh[l, d]))[:, None])[:, 0]
                        for l in (0, 1) for d in (0, 1)]).astype(BF16)[None],
        "wh1": _prep_mlp_in_x2(np.asarray(W_h1)),
        "wh2": _prep_mlp_in_h(np.asarray(W_h2)),
        "wd1": _prep_mlp_in_x2(np.asarray(W_d1)),
        "wd2": _prep_mlp_in_h(np.asarray(W_d2)),
        "wbi": _prep_wbi(np.asarray(W_bi)),
        "mb": np.stack([np.pad(np.asarray(b, np.float32), (0, 112))
                        for b in (b_h1, b_h2, b_d1, b_d2)])[None],
    }

    in_maps = []
    for k in range(N_CORES):
        lo, hi = k * VSH, (k + 1) * VSH
        shard = np.zeros((VSH + 1, WCOLS), np.float32)
        shard[:VSH] = wtab_full[lo:hi]
        local = np.where((widx_g >= lo) & (widx_g < hi), widx_g - lo, VSH)
        m = dict(base)
        m["wtab"] = shard.astype(BF16)
        m["widx"] = _wrap_idx(local)
        in_maps.append(m)
    return in_maps


def kernel(**inputs):
    in_maps = _prep(**inputs)
    nc = _get_nc(float(np.asarray(inputs["b_bi"]).reshape(-1)[0]))
    res = run_bass_kernel_spmd(nc, in_maps, list(range(N_CORES)))
    S_dev = res.results[0]["out"]            # rows/cols in parity-blocked order
    order = _pos_order()
    inv = np.empty(SEQ, np.int64)
    inv[order] = np.arange(SEQ)              # inv[t] = column position of token t
    S = S_dev[np.ix_(inv, inv)].astype(np.float32)
    return S


if __name__ == "__main__":
    print("kernel module OK; build test:", _get_nc(0.0) is not None)

